# revision 7
# baseline (speedup 1.0000x reference)
"""DeepHough (histogram binning over Hough rho bins) Trainium2 kernel.

Math: out[n,c,a,r] = sum over pixels (y,x) with rho_index[a,y,x]==r of feat[n,c,y,x].

Strategy (per NeuronCore, data-parallel over N: core c handles image n=c, all
C=128 channels):
  - The rho index table is static. For each (angle-group g of 4 angles, image
    row y) the 0/1 one-hot voting mask M[x, (a,r)] (128 x 480) is generated
    on-chip by the Vector engine: is_equal(bin[x,a] broadcast over r,
    iota[r] broadcast over a).
  - The PE array accumulates psum[nc, (a,r)] += featT[y][x, nc].T @ M over all
    128 rows y (PSUM fp32 accumulation), i.e. a one-hot matmul realization of
    the scatter-add. 4 angles * 120 rho = 480 fp32 fits one PSUM bank.
  - feat is fed as bf16 (stationary operand must be <= 2 bytes); accumulation
    is exact fp32 on top of bf16-quantized inputs.

Everything is hardcoded for feat (8,128,128,128) fp32, numangle=numrho=120,
8 cores.
"""

import os
import sys

import numpy as np

sys.path.insert(0, "/opt/trn_rl_repo")

import ml_dtypes

import concourse.bass as bass
from concourse import bacc
import concourse.mybir as mybir
import concourse.tile as tile
from concourse.bass_utils import run_bass_kernel_spmd

N, C, H, W = 8, 128, 128, 128
A, R = 120, 120
GA = 4            # angles per PSUM accumulation group (4*120 fp32 = 1 bank)
NCORES = 8

BF16 = mybir.dt.bfloat16
F32 = mybir.dt.float32


def _rho_index() -> np.ndarray:
    """Static [A, H, W] int32 table, identical math to the reference."""
    irho = int(np.sqrt(H * H + W * W) + 1) / float(R - 1)
    itheta = np.pi / A
    ang = np.arange(A) * itheta
    tab_cos = (np.cos(ang) / irho).astype(np.float32)
    tab_sin = (np.sin(ang) / irho).astype(np.float32)
    x = (np.arange(W) - W // 2).astype(np.float32)
    y = (np.arange(H) - H // 2).astype(np.float32)
    r = np.round(
        x[None, None, :] * tab_cos[:, None, None]
        + y[None, :, None] * tab_sin[:, None, None]
    ).astype(np.int32)
    r += R // 2
    return np.clip(r, 0, R - 1)


def _build(num_groups: int = A // GA) -> bass.Bass:
    nc = bacc.Bacc()
    featT_d = nc.declare_dram_parameter("featT", [W, H, C], BF16, isOutput=False)
    # tables = bins [W, A*H] ++ iota [W, R] concatenated along the free dim so
    # one DMA (one semaphore) covers both; keeps the first consumer's wait
    # count within the ISA limit.
    tables_d = nc.declare_dram_parameter("tables", [W, A * H + R], BF16, isOutput=False)
    out_d = nc.declare_dram_parameter("out", [C, num_groups * GA * R], F32, isOutput=True)

    with tile.TileContext(nc) as tc:
        with (
            tc.tile_pool(name="inp", bufs=1) as inp,
            tc.tile_pool(name="masks", bufs=6) as mp,
            tc.tile_pool(name="psum", bufs=2, space="PSUM") as pp,
        ):
            featT = inp.tile([W, H, C], BF16)
            tables = inp.tile([W, A * H + R], BF16)
            outsb = inp.tile([C, num_groups * GA * R], F32)

            nc.sync.dma_start(featT[:], featT_d[:])
            nc.sync.dma_start(tables[:], tables_d[:])
            bins = tables[:, : A * H].rearrange("p (a y) -> p a y", a=A)
            iota = tables[:, A * H :].rearrange("p (one r) -> p one r", one=1)

            for g in range(num_groups):
                ps = pp.tile([C, GA * R], F32)
                for y in range(H):
                    m = mp.tile([W, GA, R], BF16)
                    # scalar_tensor_tensor instead of tensor_tensor: identical
                    # math (op0=bypass), but lowers to the TensorScalarPtr ISA
                    # encoding which supports >1 sync-wait slot (plain TT
                    # chokes in walrus codegen once Tile attaches 2 waits).
                    nc.vector.scalar_tensor_tensor(
                        out=m[:],
                        in0=bins[:, g * GA : (g + 1) * GA, y : y + 1].broadcast_to(
                            [W, GA, R]
                        ),
                        scalar=0.0,
                        in1=iota[:, 0:1, :].broadcast_to([W, GA, R]),
                        op0=mybir.AluOpType.bypass,
                        op1=mybir.AluOpType.is_equal,
                    )
                    nc.tensor.matmul(
                        ps[:],
                        featT[:, y, :],
                        m[:].rearrange("p a r -> p (a r)"),
                        start=(y == 0),
                        stop=(y == H - 1),
                    )
                # Eviction on DVE (not ACT): the next group's first matmul then
                # depends on mask-gen AND psum-slot-release through the SAME
                # (DVE) semaphore, which Tile merges into one wait — walrus
                # here allows only one sync-wait per instruction.
                nc.vector.tensor_copy(outsb[:, g * GA * R : (g + 1) * GA * R], ps[:])

            ncols = num_groups * GA * R
            nsl = 4 if ncols % 4 == 0 else 1
            for s in range(nsl):
                sl = slice(s * (ncols // nsl), (s + 1) * (ncols // nsl))
                nc.sync.dma_start(out_d[:, sl], outsb[:, sl])

    _strip_redundant_self_waits(nc)
    nc.compile()
    nc.finalize()
    return nc


_ENGINE_SEM_PREFIX = {
    mybir.EngineType.DVE: "DVE_",
    mybir.EngineType.Pool: "POOL_",
    mybir.EngineType.Activation: "ACT_",
}


def _strip_redundant_self_waits(nc: bass.Bass) -> None:
    """Drop same-engine semaphore waits from multi-wait elementwise ops.

    Tile's wait emission is not transitively minimal: when a pool slot is
    reused it emits both the reader's (PE) wait and a same-engine WAW wait.
    The DVE/ACT/Pool elementwise ISA encodings only have ONE sync-wait slot,
    so walrus codegen dies with "Too many sync wait commands". The
    same-engine wait is redundant: engines execute and write in order, and
    the reader's wait transitively implies the earlier same-engine write
    completed (the reader waited on it).
    """
    for inst in nc.inst_map.values():
        si = inst.sync_info
        if si is None or len(si.on_wait) <= 1:
            continue
        eng = getattr(inst, "engine", None)
        pref = _ENGINE_SEM_PREFIX.get(eng)
        if pref is None:
            continue
        kept = [w for w in si.on_wait if not w.ant_name.startswith(pref)]
        if kept and len(kept) < len(si.on_wait):
            si.on_wait = kept


_NC = None


def _get_nc() -> bass.Bass:
    global _NC
    if _NC is None:
        _NC = _build()
    return _NC


def _prep_inputs(feat: np.ndarray) -> list[dict[str, np.ndarray]]:
    ridx = _rho_index()  # [a, y, x]
    bins_xay = ridx.transpose(2, 0, 1).reshape(W, A * H)  # [x, a*y]
    iota = np.broadcast_to(np.arange(R, dtype=np.int64), (W, R))
    tables = np.ascontiguousarray(
        np.concatenate([bins_xay, iota], axis=1).astype(np.float32)
    ).astype(ml_dtypes.bfloat16)
    in_maps = []
    for c in range(NCORES):
        featT = np.ascontiguousarray(feat[c].transpose(2, 1, 0)).astype(
            ml_dtypes.bfloat16
        )  # [x, y, nc]
        in_maps.append({"featT": featT, "tables": tables})
    return in_maps


def run(feat: np.ndarray, trace: bool = False):
    feat = np.asarray(feat, dtype=np.float32)
    assert feat.shape == (N, C, H, W), feat.shape
    res = run_bass_kernel_spmd(
        _get_nc(), _prep_inputs(feat), core_ids=list(range(NCORES)), trace=trace
    )
    out = np.stack(
        [res.results[c]["out"].reshape(C, A, R) for c in range(NCORES)]
    ).astype(np.float32)
    return out, res


def kernel(feat, numangle, numrho) -> np.ndarray:
    assert int(numangle) == A and int(numrho) == R
    out, _ = run(np.asarray(feat))
    return out
